# revision 14
# baseline (speedup 1.0000x reference)
"""Distributed embedding lookup (bag gather + masked mean) on 8 Trainium2 cores.

Strategy: data-parallel over the batch; each core keeps a full table replica
in HBM and handles 512 of 4096 batch rows (13312 slots).

Gather path: the table is viewed as 31 banks of <=32768 rows so bank-local row
ids fit int16, which unlocks the high-rate SWDGE gather (InstDMAGatherAnt,
~0.34ns/descriptor) instead of the generic indirect DMA (1 index per dest
partition, ~1us/instruction). The host does index routing only: it buckets
each core's valid (key, slot) pairs by bank and emits per-bank int16 local row
ids plus int16 slot ids, padded with junk rows aimed at a dump slot.

Combine path: per-bank gathered rows are recombined by slot with SBUF-dest
dma_scatter_add (CCE add, parity-split accumulators): token idx = slot id maps
to partition slot%128, free group (slot//128)//2, parity slot//128 & 1 --
exactly the [128, tile] output marshaling. The device then scales each slot by
1/max(count,1) (counts reduced from the mask on-device) and stores the result.
"""

import numpy as np

# Problem constants (hardcoded per harness contract).
B, S, N, E, V = 4096, 26, 10, 64, 1_000_000
NCORES = 8
BL = B // NCORES              # 512 batch rows per core
SL = BL * S                   # 13312 slots per core
P = 128
NT = SL // P                  # 104 tiles of 128 slots
BANK = 32768                  # rows per bank (int16 local ids)
NBANK = (V + BANK - 1) // BANK          # 31
DUMP_SLOT = 13440             # tile 105 (odd parity, group 52): trash row
NGRP = NT // 2 + 1            # 53 free-dim groups per parity accumulator
# Per-bank streams are split into within-bank occurrence subsegments so each
# scatter_add call has slot-unique indices (the CCE add races on intra-call
# duplicate targets). MIN_SUBCAPS pads the w-th occurrence counts.
MIN_SUBCAPS = (2560, 384, 128)

_STATE = {}


def _build_nc(subcaps):
    import concourse.bass as bass
    import concourse.bacc as bacc
    import concourse.mybir as mybir
    import concourse.tile as tile

    f32, i32, i16 = mybir.dt.float32, mybir.dt.int32, mybir.dt.int16
    cap = sum(subcaps)
    cw = cap // 16                        # idx columns per bank (wrapped)

    nc = bacc.Bacc("TRN2", target_bir_lowering=False, debug=False,
                   num_devices=NCORES)
    gidx_t = nc.declare_dram_parameter("gidx_t", [P, NBANK * cw], i16,
                                       isOutput=False)
    sidx_t = nc.declare_dram_parameter("sidx_t", [P, NBANK * cw], i16,
                                       isOutput=False)
    mask_t = nc.declare_dram_parameter("mask_t", [P, NT * N], i32,
                                       isOutput=False)
    table_t = nc.declare_dram_parameter("table_t", [V, E], f32,
                                        isOutput=False)
    out_t = nc.declare_dram_parameter("out_t", [P, NT * E], f32,
                                      isOutput=True)

    with tile.TileContext(nc) as tc:
        with (
            tc.tile_pool(name="persist", bufs=1) as persist,
            tc.tile_pool(name="gather", bufs=3) as gpool,
        ):
            gidx_sb = persist.tile([P, NBANK * cw], i16)
            sidx_sb = persist.tile([P, NBANK * cw], i16)
            mask_sb = persist.tile([P, NT * N], i32)
            counts_i = persist.tile([P, NT], i32)
            counts_f = persist.tile([P, NT], f32)
            recip = persist.tile([P, NT], f32)
            acc_ev = persist.tile([P, NGRP * E], f32)
            acc_od = persist.tile([P, NGRP * E], f32)
            ostage = persist.tile([P, NT * E], f32)

            nc.sync.dma_start(out=gidx_sb[:], in_=gidx_t[:])
            nc.sync.dma_start(out=sidx_sb[:], in_=sidx_t[:])
            nc.sync.dma_start(out=mask_sb[:], in_=mask_t[:])

            nc.vector.memset(acc_ev[:], 0.0)
            nc.vector.memset(acc_od[:], 0.0)

            with nc.allow_low_precision(reason="int32 sum of 10 0/1 values"):
                nc.vector.tensor_reduce(
                    out=counts_i[:],
                    in_=mask_sb[:].rearrange("p (t n) -> p t n", n=N),
                    axis=mybir.AxisListType.X,
                    op=mybir.AluOpType.add,
                )
            nc.vector.tensor_copy(out=counts_f[:], in_=counts_i[:])
            nc.vector.tensor_scalar_max(out=counts_f[:], in0=counts_f[:],
                                        scalar1=1.0)
            nc.vector.reciprocal(out=recip[:], in_=counts_f[:])

            cols = cap // P                  # gathered free columns per bank
            for b in range(NBANK):
                rows = min(BANK, V - b * BANK)
                gts = gpool.tile([P, cols * E], mybir.dt.float32, tag="gts")
                nc.gpsimd.dma_gather(
                    out_ap=gts[:].rearrange("p (c e) -> p c e", e=E),
                    in_ap=table_t[b * BANK:b * BANK + rows, :],
                    idxs_ap=gidx_sb[:, b * cw:(b + 1) * cw],
                    num_idxs=cap,
                    num_idxs_reg=cap,
                    elem_size=E,
                    single_packet=False,
                )
                # one scatter per within-bank occurrence subsegment: indices
                # inside a call are slot-unique, so CCE adds never collide.
                off = 0
                for sc in subcaps:
                    nc.gpsimd.dma_scatter_add(
                        out_ap=acc_ev[:],
                        out_ap_other=acc_od[:],
                        in_ap=gts[:, (off // P) * E:((off + sc) // P) * E]
                        .rearrange("p (c e) -> p c e", e=E),
                        idxs_ap=sidx_sb[:, b * cw + off // 16:
                                        b * cw + (off + sc) // 16],
                        num_idxs=sc,
                        num_idxs_reg=sc,
                        elem_size=E,
                        sbuf_tokens_per_rank=P,
                        parity_reg=0,
                        single_packet=False,
                    )
                    off += sc

            for t in range(NT):
                src = acc_ev if t % 2 == 0 else acc_od
                gidx = t // 2
                nc.vector.tensor_scalar_mul(
                    out=ostage[:, t * E:(t + 1) * E],
                    in0=src[:, gidx * E:(gidx + 1) * E],
                    scalar1=recip[:, t:t + 1])
            nc.sync.dma_start(out=out_t[:], in_=ostage[:])
    nc.compile()
    return nc


def _make_runner(nc):
    import jax
    import concourse.mybir as mybir
    from concourse import bass2jax
    from jax.sharding import Mesh, PartitionSpec
    from jax.experimental.shard_map import shard_map

    bass2jax.install_neuronx_cc_hook()

    in_names, out_names, out_avals, zero_shapes = [], [], [], []
    partition_name = (nc.partition_id_tensor.name
                      if nc.partition_id_tensor else None)
    for alloc in nc.m.functions[0].allocations:
        if not isinstance(alloc, mybir.MemoryLocationSet):
            continue
        name = alloc.memorylocations[0].name
        if alloc.kind == "ExternalInput":
            if name != partition_name:
                in_names.append(name)
        elif alloc.kind == "ExternalOutput":
            out_names.append(name)
            shape = tuple(alloc.tensor_shape)
            dtype = mybir.dt.np(alloc.dtype)
            out_avals.append(jax.core.ShapedArray(shape, dtype))
            zero_shapes.append((shape, dtype))
    n_params = len(in_names)
    n_outs = len(out_avals)
    all_in_names = list(in_names) + list(out_names)
    if partition_name is not None:
        all_in_names.append(partition_name)
    donate = tuple(range(n_params, n_params + n_outs))

    def _body(*args):
        operands = list(args)
        if partition_name is not None:
            operands.append(bass2jax.partition_id_tensor())
        outs = bass2jax._bass_exec_p.bind(
            *operands,
            out_avals=tuple(out_avals),
            in_names=tuple(all_in_names),
            out_names=tuple(out_names),
            lowering_input_output_aliases=(),
            sim_require_finite=True,
            sim_require_nnan=True,
            nc=nc,
        )
        return tuple(outs)

    devices = jax.devices()[:NCORES]
    mesh = Mesh(np.asarray(devices), ("core",))
    # per-core inputs sharded by core; the table is replicated
    specs = []
    for name in in_names:
        specs.append(PartitionSpec() if name == "table_t"
                     else PartitionSpec("core"))
    in_specs = tuple(specs) + (PartitionSpec("core"),) * n_outs
    out_specs = (PartitionSpec("core"),) * len(out_names)
    fn = jax.jit(
        shard_map(_body, mesh=mesh, in_specs=in_specs, out_specs=out_specs,
                  check_rep=False),
        donate_argnums=donate, keep_unused=True,
    )
    return fn, mesh, in_names, out_names, zero_shapes


def _wrap16(arr, cap):
    """[cap] int16 -> [16, cap//16] wrapped layout, replicated to 128 rows."""
    w = arr.reshape(cap // 16, 16).T       # [16, cw]
    return np.tile(w, (8, 1))              # [128, cw]


def _bank_rank(bank, slot):
    """Per (bank, slot) occurrence rank for bank-sorted, slot-sorted streams."""
    key = bank * (SL + 1) + slot
    order = np.argsort(key, kind="stable")
    ks = key[order]
    # occurrence index within equal runs of ks
    first = np.concatenate(([0], np.flatnonzero(np.diff(ks)) + 1))
    runidx = np.arange(len(ks)) - np.repeat(first, np.diff(
        np.concatenate((first, [len(ks)]))))
    w = np.empty(len(ks), np.int64)
    w[order] = runidx
    return w


def _percore_lists(keys, mask, c):
    slots_all = np.arange(SL, dtype=np.int64).repeat(N)
    kc = np.asarray(keys[c * BL:(c + 1) * BL]).reshape(-1)
    mc = np.asarray(mask[c * BL:(c + 1) * BL]).reshape(-1)
    valid = mc != 0
    k = kc[valid].astype(np.int64)
    s = slots_all[valid]
    bank = k // BANK
    w = _bank_rank(bank, s)
    return k, s, bank, w, mc


def marshal_inputs(keys, mask, subcaps):
    """Bucket valid (key, slot) pairs by bank; split each bank stream into
    within-bank occurrence subsegments (slot-unique per subsegment)."""
    cap = sum(subcaps)
    cw = cap // 16
    nsub = len(subcaps)
    suboff = np.concatenate(([0], np.cumsum(subcaps)))
    gidx_g = np.empty((NCORES * P, NBANK * cw), np.int16)
    sidx_g = np.empty((NCORES * P, NBANK * cw), np.int16)
    mask_g = np.empty((NCORES * P, NT * N), np.int32)
    for c in range(NCORES):
        k, s, bank, w, mc = _percore_lists(keys, mask, c)
        if w.max(initial=0) >= nsub:
            raise OverflowError(f"occurrence {w.max()} >= {nsub} subsegments")
        gi = np.zeros(NBANK * cap, np.int16)
        si = np.full(NBANK * cap, DUMP_SLOT, np.int16)
        # position rows: for each (bank, w) pair, running index
        bw = bank * nsub + w
        counts = np.bincount(bw, minlength=NBANK * nsub)
        cmax = counts.reshape(NBANK, nsub).max(axis=0)
        if (cmax > np.asarray(subcaps)).any():
            raise OverflowError(f"subcap overflow: {cmax} vs {subcaps}")
        offs = np.concatenate(([0], np.cumsum(counts)))
        order = np.argsort(bw, kind="stable")
        run_sorted = np.arange(len(k)) - offs[bw[order]]
        run = np.empty(len(k), np.int64)
        run[order] = run_sorted
        # run[i]: index of row i within its (bank, w) bucket
        pos = bank * cap + suboff[w] + run
        gi[pos] = (k % BANK).astype(np.int16)
        si[pos] = s.astype(np.int16)
        for b in range(NBANK):
            gidx_g[c * P:(c + 1) * P, b * cw:(b + 1) * cw] = _wrap16(
                gi[b * cap:(b + 1) * cap], cap)
            sidx_g[c * P:(c + 1) * P, b * cw:(b + 1) * cw] = _wrap16(
                si[b * cap:(b + 1) * cap], cap)
        mask_g[c * P:(c + 1) * P] = (
            mc.reshape(SL, N).reshape(NT, P, N).transpose(1, 0, 2)
            .reshape(P, NT * N).astype(np.int32))
    return gidx_g, sidx_g, mask_g


def needed_subcaps(keys, mask):
    mx = np.zeros(16, np.int64)
    nsub_max = 0
    for c in range(NCORES):
        k, s, bank, w, mc = _percore_lists(keys, mask, c)
        nsub_max = max(nsub_max, int(w.max(initial=0)) + 1)
        counts = np.bincount(bank * 16 + w, minlength=NBANK * 16)
        mx = np.maximum(mx, counts.reshape(NBANK, 16).max(axis=0))
    subcaps = []
    for i in range(nsub_max):
        need = ((int(mx[i]) + 127) // 128) * 128
        base = MIN_SUBCAPS[i] if i < len(MIN_SUBCAPS) else 128
        subcaps.append(max(base, need))
    return tuple(subcaps)


def unmarshal_output(out_g):
    out = np.empty((B, S, E), np.float32)
    for c in range(NCORES):
        oc = np.asarray(out_g[c * P:(c + 1) * P])
        out[c * BL:(c + 1) * BL] = (
            oc.reshape(P, NT, E).transpose(1, 0, 2).reshape(BL, S, E))
    return out


def _get_state(subcaps):
    if _STATE.get("subcaps") != subcaps:
        nc = _build_nc(subcaps)
        fn, mesh, in_names, out_names, zero_shapes = _make_runner(nc)
        _STATE.update(subcaps=subcaps, nc=nc, fn=fn, mesh=mesh,
                      in_names=in_names, out_names=out_names,
                      zero_shapes=zero_shapes, table_key=None)
    return _STATE


def kernel(keys, mask, table):
    import jax
    from jax.sharding import NamedSharding, PartitionSpec

    subcaps = needed_subcaps(keys, mask)
    st = _get_state(subcaps)
    gidx_g, sidx_g, mask_g = marshal_inputs(keys, mask, subcaps)

    tkey = id(table)
    if st.get("table_key") != tkey:
        st["table_dev"] = jax.device_put(
            np.ascontiguousarray(np.asarray(table, dtype=np.float32)),
            NamedSharding(st["mesh"], PartitionSpec()))
        st["table_key"] = tkey

    inputs = {"gidx_t": gidx_g, "sidx_t": sidx_g, "mask_t": mask_g,
              "table_t": st["table_dev"]}
    args = [inputs[name] for name in st["in_names"]]
    zshape, zdtype = st["zero_shapes"][0]
    zeros_out = np.zeros((NCORES * zshape[0], *zshape[1:]), zdtype)
    outs = st["fn"](*args, zeros_out)
    out_g = np.asarray(jax.block_until_ready(outs[0]))
    return unmarshal_output(out_g)


# revision 15
# speedup vs baseline: 1.5617x; 1.5617x over previous
"""Distributed embedding lookup (bag gather + masked mean) on 8 Trainium2 cores.

Strategy: data-parallel over the batch. Each core holds a full replica of the
embedding table in its HBM and processes 512 of the 4096 batch rows:
  - host marshals keys/mask into a [128 partitions, 104 tiles * 10 nnz] layout
  - device remaps masked-out keys to a sentinel row (appended zero row in the
    table) so the gather itself zeroes invalid entries
  - one indirect DMA per 1024 slots gathers 10240 embedding rows into SBUF
  - DVE reduces each slot's 10 rows (tree add) and scales by 1/max(count,1)
No collectives needed: replication beats the key%8 model-parallel split since
the 256 MB table fits per-core and reduce-scatter traffic is avoided.
"""

import numpy as np

# Problem constants (hardcoded per harness contract).
B, S, N, E, V = 4096, 26, 10, 64, 1_000_000
NCORES = 8
BL = B // NCORES              # 512 batch rows per core
SL = BL * S                   # 13312 slots per core
P = 128                       # SBUF partitions
NT = SL // P                  # 104 tiles of 128 slots
GT = 8                        # tiles per gather super-tile
NSUP = NT // GT               # 13 super-tiles
KPS = GT * N                  # 80 keys per partition per super-tile
VPAD = V + 8                  # table padded with 8 zero rows; sentinel = V

_STATE = {}


def _build_nc():
    import concourse.bacc as bacc
    import concourse.mybir as mybir
    import concourse.tile as tile

    nc = bacc.Bacc("TRN2", target_bir_lowering=False, debug=False,
                   num_devices=NCORES)
    keys_t = nc.declare_dram_parameter("keys_t", [P, NT * N], mybir.dt.int32,
                                       isOutput=False)
    mask_t = nc.declare_dram_parameter("mask_t", [P, NT * N], mybir.dt.int32,
                                       isOutput=False)
    table_t = nc.declare_dram_parameter("table_t", [VPAD, E], mybir.dt.float32,
                                        isOutput=False)
    out_t = nc.declare_dram_parameter("out_t", [P, NT * E], mybir.dt.float32,
                                      isOutput=True)

    import concourse.bass as bass
    f32 = mybir.dt.float32
    i32 = mybir.dt.int32

    with tile.TileContext(nc) as tc:
        with (
            tc.tile_pool(name="persist", bufs=1) as persist,
            tc.tile_pool(name="gather", bufs=3) as gpool,
            tc.tile_pool(name="tmp", bufs=4) as tpool,
            tc.tile_pool(name="outp", bufs=3) as opool,
        ):
            keys_sb = persist.tile([P, NT * N], i32)
            mask_sb = persist.tile([P, NT * N], i32)
            adj_sb = persist.tile([P, NT * N], i32)
            counts_i = persist.tile([P, NT], i32)
            counts_f = persist.tile([P, NT], f32)
            recip = persist.tile([P, NT], f32)

            nc.sync.dma_start(out=keys_sb[:], in_=keys_t[:])
            nc.sync.dma_start(out=mask_sb[:], in_=mask_t[:])

            # counts per slot = sum of mask over the 10 nnz positions
            with nc.allow_low_precision(reason="int32 sum of 10 0/1 values"):
                nc.vector.tensor_reduce(
                    out=counts_i[:],
                    in_=mask_sb[:].rearrange("p (t n) -> p t n", n=N),
                    axis=mybir.AxisListType.X,
                    op=mybir.AluOpType.add,
                )
            nc.vector.tensor_copy(out=counts_f[:], in_=counts_i[:])
            nc.vector.tensor_scalar_max(out=counts_f[:], in0=counts_f[:],
                                        scalar1=1.0)
            nc.vector.reciprocal(out=recip[:], in_=counts_f[:])

            # adj = mask ? key : V  (V indexes the appended zero row)
            # adj = (key - V) * mask + V
            nc.vector.tensor_scalar_add(out=adj_sb[:], in0=keys_sb[:],
                                        scalar1=-V)
            nc.vector.tensor_tensor(out=adj_sb[:], in0=adj_sb[:],
                                    in1=mask_sb[:], op=mybir.AluOpType.mult)
            nc.vector.tensor_scalar_add(out=adj_sb[:], in0=adj_sb[:],
                                        scalar1=V)

            for g in range(NSUP):
                gt = gpool.tile([P, GT * N * E], f32)
                # HW indirect DMA consumes ONE index per dest partition row:
                # issue one gather per key column, [P,1] idx -> [P,E] dest.
                for j in range(KPS):
                    nc.gpsimd.indirect_dma_start(
                        out=gt[:, j * E:(j + 1) * E],
                        out_offset=None,
                        in_=table_t[:],
                        in_offset=bass.IndirectOffsetOnAxis(
                            ap=adj_sb[:, g * KPS + j:g * KPS + j + 1], axis=0),
                    )
                osup = opool.tile([P, GT * E], f32)
                for i in range(GT):
                    tt = g * GT + i
                    sl = gt[:, i * N * E:(i + 1) * N * E]
                    t320 = tpool.tile([P, 5 * E], f32)
                    t128 = tpool.tile([P, 2 * E], f32)
                    t64 = tpool.tile([P, E], f32)
                    nc.vector.tensor_add(out=t320[:], in0=sl[:, 0:5 * E],
                                         in1=sl[:, 5 * E:10 * E])
                    nc.vector.tensor_add(out=t128[:], in0=t320[:, 0:2 * E],
                                         in1=t320[:, 2 * E:4 * E])
                    nc.vector.tensor_add(out=t64[:], in0=t128[:, 0:E],
                                         in1=t128[:, E:2 * E])
                    nc.vector.tensor_add(out=t64[:], in0=t64[:],
                                         in1=t320[:, 4 * E:5 * E])
                    nc.vector.tensor_scalar_mul(
                        out=osup[:, i * E:(i + 1) * E], in0=t64[:],
                        scalar1=recip[:, tt:tt + 1])
                nc.sync.dma_start(out=out_t[:, g * GT * E:(g + 1) * GT * E],
                                  in_=osup[:])
    nc.compile()
    return nc


def _make_runner(nc):
    import jax
    import concourse.mybir as mybir
    from concourse import bass2jax
    from jax.sharding import Mesh, PartitionSpec
    from jax.experimental.shard_map import shard_map

    bass2jax.install_neuronx_cc_hook()

    in_names, out_names, out_avals, zero_shapes = [], [], [], []
    partition_name = (nc.partition_id_tensor.name
                      if nc.partition_id_tensor else None)
    for alloc in nc.m.functions[0].allocations:
        if not isinstance(alloc, mybir.MemoryLocationSet):
            continue
        name = alloc.memorylocations[0].name
        if alloc.kind == "ExternalInput":
            if name != partition_name:
                in_names.append(name)
        elif alloc.kind == "ExternalOutput":
            out_names.append(name)
            shape = tuple(alloc.tensor_shape)
            dtype = mybir.dt.np(alloc.dtype)
            out_avals.append(jax.core.ShapedArray(shape, dtype))
            zero_shapes.append((shape, dtype))
    n_params = len(in_names)
    n_outs = len(out_avals)
    all_in_names = list(in_names) + list(out_names)
    if partition_name is not None:
        all_in_names.append(partition_name)
    donate = tuple(range(n_params, n_params + n_outs))

    def _body(*args):
        operands = list(args)
        if partition_name is not None:
            operands.append(bass2jax.partition_id_tensor())
        outs = bass2jax._bass_exec_p.bind(
            *operands,
            out_avals=tuple(out_avals),
            in_names=tuple(all_in_names),
            out_names=tuple(out_names),
            lowering_input_output_aliases=(),
            sim_require_finite=True,
            sim_require_nnan=True,
            nc=nc,
        )
        return tuple(outs)

    devices = jax.devices()[:NCORES]
    mesh = Mesh(np.asarray(devices), ("core",))
    # keys/mask sharded by core; table replicated; donated output sharded
    in_specs = (PartitionSpec("core"), PartitionSpec("core"), PartitionSpec(),
                PartitionSpec("core"))
    out_specs = (PartitionSpec("core"),)
    fn = jax.jit(
        shard_map(_body, mesh=mesh, in_specs=in_specs, out_specs=out_specs,
                  check_rep=False),
        donate_argnums=donate, keep_unused=True,
    )
    return fn, mesh, in_names, out_names, zero_shapes


def _get_state():
    if "fn" not in _STATE:
        nc = _build_nc()
        fn, mesh, in_names, out_names, zero_shapes = _make_runner(nc)
        _STATE.update(nc=nc, fn=fn, mesh=mesh, in_names=in_names,
                      out_names=out_names, zero_shapes=zero_shapes)
    return _STATE


def _marshal_percore(arr_c):
    """[BL, S, N] -> [P, NT*N] partition-major tile layout."""
    return (arr_c.reshape(SL, N).reshape(NT, P, N).transpose(1, 0, 2)
            .reshape(P, NT * N))


def marshal_inputs(keys, mask):
    keys_g = np.empty((NCORES * P, NT * N), np.int32)
    mask_g = np.empty((NCORES * P, NT * N), np.int32)
    for c in range(NCORES):
        sl = slice(c * BL, (c + 1) * BL)
        keys_g[c * P:(c + 1) * P] = _marshal_percore(
            np.ascontiguousarray(keys[sl], dtype=np.int32))
        mask_g[c * P:(c + 1) * P] = _marshal_percore(
            mask[sl].astype(np.int32))
    return keys_g, mask_g


def pad_table(table):
    table_ext = np.zeros((VPAD, E), np.float32)
    table_ext[:V] = table
    return table_ext


def unmarshal_output(out_g):
    """[NCORES*P, NT*E] -> [B, S, E]"""
    out = np.empty((B, S, E), np.float32)
    for c in range(NCORES):
        oc = np.asarray(out_g[c * P:(c + 1) * P])  # [P, NT*E]
        out[c * BL:(c + 1) * BL] = (
            oc.reshape(P, NT, E).transpose(1, 0, 2).reshape(BL, S, E))
    return out


def kernel(keys, mask, table):
    import jax
    from jax.sharding import NamedSharding, PartitionSpec

    st = _get_state()
    keys_g, mask_g = marshal_inputs(np.asarray(keys), np.asarray(mask))

    tkey = id(table)
    if _STATE.get("table_key") != tkey:
        table_ext = pad_table(np.asarray(table, dtype=np.float32))
        _STATE["table_dev"] = jax.device_put(
            table_ext, NamedSharding(st["mesh"], PartitionSpec()))
        _STATE["table_key"] = tkey

    zshape, zdtype = st["zero_shapes"][0]
    zeros_out = np.zeros((NCORES * zshape[0], *zshape[1:]), zdtype)
    outs = st["fn"](keys_g, mask_g, _STATE["table_dev"], zeros_out)
    out_g = np.asarray(jax.block_until_ready(outs[0]))
    return unmarshal_output(out_g)


# revision 16
# speedup vs baseline: 1.5653x; 1.0023x over previous
"""Distributed embedding lookup (bag gather + masked mean) on 8 Trainium2 cores.

Strategy: data-parallel over the batch. Each core holds a full replica of the
embedding table in its HBM and processes 512 of the 4096 batch rows:
  - host marshals keys/mask into a [128 partitions, 104 tiles * 10 nnz] layout
  - device remaps masked-out keys to a sentinel row (appended zero row in the
    table) so the gather itself zeroes invalid entries
  - one indirect DMA per 1024 slots gathers 10240 embedding rows into SBUF
  - DVE reduces each slot's 10 rows (tree add) and scales by 1/max(count,1)
No collectives needed: replication beats the key%8 model-parallel split since
the 256 MB table fits per-core and reduce-scatter traffic is avoided.
"""

import numpy as np

# Problem constants (hardcoded per harness contract).
B, S, N, E, V = 4096, 26, 10, 64, 1_000_000
NCORES = 8
BL = B // NCORES              # 512 batch rows per core
SL = BL * S                   # 13312 slots per core
P = 128                       # SBUF partitions
NT = SL // P                  # 104 tiles of 128 slots
GT = 8                        # tiles per gather super-tile
NSUP = NT // GT               # 13 super-tiles
KPS = GT * N                  # 80 keys per partition per super-tile
VPAD = V + 8                  # table padded with 8 zero rows; sentinel = V

_STATE = {}


def _build_nc():
    import concourse.bacc as bacc
    import concourse.mybir as mybir
    import concourse.tile as tile

    nc = bacc.Bacc("TRN2", target_bir_lowering=False, debug=False,
                   num_devices=NCORES)
    keys_t = nc.declare_dram_parameter("keys_t", [P, NT * N], mybir.dt.int32,
                                       isOutput=False)
    mask_t = nc.declare_dram_parameter("mask_t", [P, NT * N], mybir.dt.int32,
                                       isOutput=False)
    table_t = nc.declare_dram_parameter("table_t", [VPAD, E], mybir.dt.float32,
                                        isOutput=False)
    out_t = nc.declare_dram_parameter("out_t", [P, NT * E], mybir.dt.float32,
                                      isOutput=True)

    import concourse.bass as bass
    f32 = mybir.dt.float32
    i32 = mybir.dt.int32

    with tile.TileContext(nc) as tc:
        with (
            tc.tile_pool(name="persist", bufs=1) as persist,
            tc.tile_pool(name="gather", bufs=5) as gpool,
            tc.tile_pool(name="tmp", bufs=8) as tpool,
            tc.tile_pool(name="outp", bufs=4) as opool,
        ):
            keys_sb = persist.tile([P, NT * N], i32)
            mask_sb = persist.tile([P, NT * N], i32)
            adj_sb = persist.tile([P, NT * N], i32)
            counts_i = persist.tile([P, NT], i32)
            counts_f = persist.tile([P, NT], f32)
            recip = persist.tile([P, NT], f32)

            nc.sync.dma_start(out=keys_sb[:], in_=keys_t[:])
            nc.sync.dma_start(out=mask_sb[:], in_=mask_t[:])

            # counts per slot = sum of mask over the 10 nnz positions
            with nc.allow_low_precision(reason="int32 sum of 10 0/1 values"):
                nc.vector.tensor_reduce(
                    out=counts_i[:],
                    in_=mask_sb[:].rearrange("p (t n) -> p t n", n=N),
                    axis=mybir.AxisListType.X,
                    op=mybir.AluOpType.add,
                )
            nc.vector.tensor_copy(out=counts_f[:], in_=counts_i[:])
            nc.vector.tensor_scalar_max(out=counts_f[:], in0=counts_f[:],
                                        scalar1=1.0)
            nc.vector.reciprocal(out=recip[:], in_=counts_f[:])

            # adj = mask ? key : V  (V indexes the appended zero row)
            # adj = (key - V) * mask + V
            nc.vector.tensor_scalar_add(out=adj_sb[:], in0=keys_sb[:],
                                        scalar1=-V)
            nc.vector.tensor_tensor(out=adj_sb[:], in0=adj_sb[:],
                                    in1=mask_sb[:], op=mybir.AluOpType.mult)
            nc.vector.tensor_scalar_add(out=adj_sb[:], in0=adj_sb[:],
                                        scalar1=V)

            for g in range(NSUP):
                gt = gpool.tile([P, GT * N * E], f32)
                # HW indirect DMA consumes ONE index per dest partition row:
                # issue one gather per key column, [P,1] idx -> [P,E] dest.
                for j in range(KPS):
                    nc.gpsimd.indirect_dma_start(
                        out=gt[:, j * E:(j + 1) * E],
                        out_offset=None,
                        in_=table_t[:],
                        in_offset=bass.IndirectOffsetOnAxis(
                            ap=adj_sb[:, g * KPS + j:g * KPS + j + 1], axis=0),
                    )
                osup = opool.tile([P, GT * E], f32)
                for i in range(GT):
                    tt = g * GT + i
                    sl = gt[:, i * N * E:(i + 1) * N * E]
                    t320 = tpool.tile([P, 5 * E], f32)
                    t128 = tpool.tile([P, 2 * E], f32)
                    t64 = tpool.tile([P, E], f32)
                    nc.vector.tensor_add(out=t320[:], in0=sl[:, 0:5 * E],
                                         in1=sl[:, 5 * E:10 * E])
                    nc.vector.tensor_add(out=t128[:], in0=t320[:, 0:2 * E],
                                         in1=t320[:, 2 * E:4 * E])
                    nc.vector.tensor_add(out=t64[:], in0=t128[:, 0:E],
                                         in1=t128[:, E:2 * E])
                    nc.vector.tensor_add(out=t64[:], in0=t64[:],
                                         in1=t320[:, 4 * E:5 * E])
                    nc.vector.tensor_scalar_mul(
                        out=osup[:, i * E:(i + 1) * E], in0=t64[:],
                        scalar1=recip[:, tt:tt + 1])
                nc.sync.dma_start(out=out_t[:, g * GT * E:(g + 1) * GT * E],
                                  in_=osup[:])
    nc.compile()
    return nc


def _make_runner(nc):
    import jax
    import concourse.mybir as mybir
    from concourse import bass2jax
    from jax.sharding import Mesh, PartitionSpec
    from jax.experimental.shard_map import shard_map

    bass2jax.install_neuronx_cc_hook()

    in_names, out_names, out_avals, zero_shapes = [], [], [], []
    partition_name = (nc.partition_id_tensor.name
                      if nc.partition_id_tensor else None)
    for alloc in nc.m.functions[0].allocations:
        if not isinstance(alloc, mybir.MemoryLocationSet):
            continue
        name = alloc.memorylocations[0].name
        if alloc.kind == "ExternalInput":
            if name != partition_name:
                in_names.append(name)
        elif alloc.kind == "ExternalOutput":
            out_names.append(name)
            shape = tuple(alloc.tensor_shape)
            dtype = mybir.dt.np(alloc.dtype)
            out_avals.append(jax.core.ShapedArray(shape, dtype))
            zero_shapes.append((shape, dtype))
    n_params = len(in_names)
    n_outs = len(out_avals)
    all_in_names = list(in_names) + list(out_names)
    if partition_name is not None:
        all_in_names.append(partition_name)
    donate = tuple(range(n_params, n_params + n_outs))

    def _body(*args):
        operands = list(args)
        if partition_name is not None:
            operands.append(bass2jax.partition_id_tensor())
        outs = bass2jax._bass_exec_p.bind(
            *operands,
            out_avals=tuple(out_avals),
            in_names=tuple(all_in_names),
            out_names=tuple(out_names),
            lowering_input_output_aliases=(),
            sim_require_finite=True,
            sim_require_nnan=True,
            nc=nc,
        )
        return tuple(outs)

    devices = jax.devices()[:NCORES]
    mesh = Mesh(np.asarray(devices), ("core",))
    # keys/mask sharded by core; table replicated; donated output sharded
    in_specs = (PartitionSpec("core"), PartitionSpec("core"), PartitionSpec(),
                PartitionSpec("core"))
    out_specs = (PartitionSpec("core"),)
    fn = jax.jit(
        shard_map(_body, mesh=mesh, in_specs=in_specs, out_specs=out_specs,
                  check_rep=False),
        donate_argnums=donate, keep_unused=True,
    )
    return fn, mesh, in_names, out_names, zero_shapes


def _get_state():
    if "fn" not in _STATE:
        nc = _build_nc()
        fn, mesh, in_names, out_names, zero_shapes = _make_runner(nc)
        _STATE.update(nc=nc, fn=fn, mesh=mesh, in_names=in_names,
                      out_names=out_names, zero_shapes=zero_shapes)
    return _STATE


def _marshal_percore(arr_c):
    """[BL, S, N] -> [P, NT*N] partition-major tile layout."""
    return (arr_c.reshape(SL, N).reshape(NT, P, N).transpose(1, 0, 2)
            .reshape(P, NT * N))


def marshal_inputs(keys, mask):
    keys_g = np.empty((NCORES * P, NT * N), np.int32)
    mask_g = np.empty((NCORES * P, NT * N), np.int32)
    for c in range(NCORES):
        sl = slice(c * BL, (c + 1) * BL)
        keys_g[c * P:(c + 1) * P] = _marshal_percore(
            np.ascontiguousarray(keys[sl], dtype=np.int32))
        mask_g[c * P:(c + 1) * P] = _marshal_percore(
            mask[sl].astype(np.int32))
    return keys_g, mask_g


def pad_table(table):
    table_ext = np.zeros((VPAD, E), np.float32)
    table_ext[:V] = table
    return table_ext


def unmarshal_output(out_g):
    """[NCORES*P, NT*E] -> [B, S, E]"""
    out = np.empty((B, S, E), np.float32)
    for c in range(NCORES):
        oc = np.asarray(out_g[c * P:(c + 1) * P])  # [P, NT*E]
        out[c * BL:(c + 1) * BL] = (
            oc.reshape(P, NT, E).transpose(1, 0, 2).reshape(BL, S, E))
    return out


def kernel(keys, mask, table):
    import jax
    from jax.sharding import NamedSharding, PartitionSpec

    st = _get_state()
    keys_g, mask_g = marshal_inputs(np.asarray(keys), np.asarray(mask))

    tkey = id(table)
    if _STATE.get("table_key") != tkey:
        table_ext = pad_table(np.asarray(table, dtype=np.float32))
        _STATE["table_dev"] = jax.device_put(
            table_ext, NamedSharding(st["mesh"], PartitionSpec()))
        _STATE["table_key"] = tkey

    zshape, zdtype = st["zero_shapes"][0]
    zeros_out = np.zeros((NCORES * zshape[0], *zshape[1:]), zdtype)
    outs = st["fn"](keys_g, mask_g, _STATE["table_dev"], zeros_out)
    out_g = np.asarray(jax.block_until_ready(outs[0]))
    return unmarshal_output(out_g)


# revision 18
# speedup vs baseline: 2.0810x; 1.3295x over previous
"""Distributed embedding lookup (bag gather + masked mean) on 8 Trainium2 cores.

Data-parallel over the batch; each core keeps a full table replica in HBM and
handles 512 of 4096 batch rows (13312 slots, partition-tile layout).

The gather is Pool-engine bound (~1.34us per indirect-DMA instruction, 128
rows each), so the row count per instruction column matters. Host-side index
routing compacts each slot's valid keys first:
  - static part: the first C=5 valid keys per slot -> 5*104 gather columns
    (missing keys point at an appended zero row).
  - overflow part (6th..10th valid key, ~16k rows/core): packed densely
    across partitions into per-occurrence column blocks, fetched by the same
    indirect gather, then recombined by slot id with SBUF-dest dma_scatter_add
    (CCE add, parity-split accumulators). Each call spans one occurrence
    block, so indices within a call are slot-unique (the CCE add races on
    intra-call duplicates). The accumulators are folded into the static tree
    before the 1/max(count,1) scale.
"""

import numpy as np

# Problem constants (hardcoded per harness contract).
B, S, N, E, V = 4096, 26, 10, 64, 1_000_000
NCORES = 8
BL = B // NCORES              # 512 batch rows per core
SL = BL * S                   # 13312 slots per core
P = 128
NT = SL // P                  # 104 tiles of 128 slots
C = 5                         # statically gathered keys per slot
GT = 8                        # tiles per gather super-tile
NSUP = NT // GT               # 13
VPAD = V + 8                  # zero sentinel row at index V
DUMP_SLOT = 13440             # tile 105 (odd parity, group 52): trash row
NGRP = NT // 2 + 1            # 53 free-dim groups per parity accumulator

_STATE = {}


def _build_nc(ocols):
    """ocols: tuple of per-occurrence overflow column counts."""
    import concourse.bass as bass
    import concourse.bacc as bacc
    import concourse.mybir as mybir
    import concourse.tile as tile

    f32, i32, i16 = mybir.dt.float32, mybir.dt.int32, mybir.dt.int16
    OC = sum(ocols)

    nc = bacc.Bacc("TRN2", target_bir_lowering=False, debug=False,
                   num_devices=NCORES)
    skeys_t = nc.declare_dram_parameter("skeys_t", [P, NT * C], i32,
                                        isOutput=False)
    okeys_t = nc.declare_dram_parameter("okeys_t", [P, max(OC, 1)], i32,
                                        isOutput=False)
    osid_t = nc.declare_dram_parameter("osid_t", [P, max(OC, 1) * 8], i16,
                                       isOutput=False)
    mask_t = nc.declare_dram_parameter("mask_t", [P, NT * N], i32,
                                       isOutput=False)
    table_t = nc.declare_dram_parameter("table_t", [VPAD, E], f32,
                                        isOutput=False)
    out_t = nc.declare_dram_parameter("out_t", [P, NT * E], f32,
                                      isOutput=True)

    with tile.TileContext(nc) as tc:
        with (
            tc.tile_pool(name="persist", bufs=1) as persist,
            tc.tile_pool(name="gather", bufs=4) as gpool,
            tc.tile_pool(name="tmp", bufs=8) as tpool,
            tc.tile_pool(name="outp", bufs=4) as opool,
        ):
            skeys_sb = persist.tile([P, NT * C], i32)
            okeys_sb = persist.tile([P, max(OC, 1)], i32)
            osid_sb = persist.tile([P, max(OC, 1) * 8], i16)
            mask_sb = persist.tile([P, NT * N], i32)
            counts_i = persist.tile([P, NT], i32)
            counts_f = persist.tile([P, NT], f32)
            recip = persist.tile([P, NT], f32)
            acc_ev = persist.tile([P, NGRP * E], f32)
            acc_od = persist.tile([P, NGRP * E], f32)
            ogt = persist.tile([P, max(OC, 1) * E], f32)

            nc.sync.dma_start(out=skeys_sb[:], in_=skeys_t[:])
            nc.sync.dma_start(out=okeys_sb[:], in_=okeys_t[:])
            nc.sync.dma_start(out=osid_sb[:], in_=osid_t[:])
            nc.sync.dma_start(out=mask_sb[:], in_=mask_t[:])

            nc.vector.memset(acc_ev[:], 0.0)
            nc.vector.memset(acc_od[:], 0.0)

            with nc.allow_low_precision(reason="int32 sum of 10 0/1 values"):
                nc.vector.tensor_reduce(
                    out=counts_i[:],
                    in_=mask_sb[:].rearrange("p (t n) -> p t n", n=N),
                    axis=mybir.AxisListType.X,
                    op=mybir.AluOpType.add,
                )
            nc.vector.tensor_copy(out=counts_f[:], in_=counts_i[:])
            nc.vector.tensor_scalar_max(out=counts_f[:], in0=counts_f[:],
                                        scalar1=1.0)
            nc.vector.reciprocal(out=recip[:], in_=counts_f[:])

            # overflow: gather columns, then slot-unique scatter_adds per
            # occurrence block into the parity-split accumulators.
            for c in range(OC):
                nc.gpsimd.indirect_dma_start(
                    out=ogt[:, c * E:(c + 1) * E],
                    out_offset=None,
                    in_=table_t[:],
                    in_offset=bass.IndirectOffsetOnAxis(
                        ap=okeys_sb[:, c:c + 1], axis=0),
                )
            off = 0
            for cj in ocols:
                # sub-calls of <=4 columns (512 idxs) keep single-packet mode
                # within the HW descriptor-ring limit
                for o2 in range(off, off + cj, 4):
                    cs = min(4, off + cj - o2)
                    nc.gpsimd.dma_scatter_add(
                        out_ap=acc_ev[:],
                        out_ap_other=acc_od[:],
                        in_ap=ogt[:, o2 * E:(o2 + cs) * E]
                        .rearrange("p (c e) -> p c e", e=E),
                        idxs_ap=osid_sb[:, o2 * 8:(o2 + cs) * 8],
                        num_idxs=cs * P,
                        num_idxs_reg=cs * P,
                        elem_size=E,
                        sbuf_tokens_per_rank=P,
                        parity_reg=0,
                    )
                off += cj

            # static part: C columns per tile, tree reduce + acc fold + scale
            for g in range(NSUP):
                gt = gpool.tile([P, GT * C * E], f32)
                for j in range(GT * C):
                    nc.gpsimd.indirect_dma_start(
                        out=gt[:, j * E:(j + 1) * E],
                        out_offset=None,
                        in_=table_t[:],
                        in_offset=bass.IndirectOffsetOnAxis(
                            ap=skeys_sb[:, g * GT * C + j:g * GT * C + j + 1],
                            axis=0),
                    )
                osup = opool.tile([P, GT * E], f32)
                for i in range(GT):
                    tt = g * GT + i
                    sl = gt[:, i * C * E:(i + 1) * C * E]
                    acc = acc_ev if tt % 2 == 0 else acc_od
                    aslice = acc[:, (tt // 2) * E:(tt // 2 + 1) * E]
                    t128 = tpool.tile([P, 2 * E], f32)
                    t64 = tpool.tile([P, E], f32)
                    nc.vector.tensor_add(out=t128[:], in0=sl[:, 0:2 * E],
                                         in1=sl[:, 2 * E:4 * E])
                    nc.vector.tensor_add(out=t64[:], in0=t128[:, 0:E],
                                         in1=t128[:, E:2 * E])
                    nc.vector.tensor_add(out=t64[:], in0=t64[:],
                                         in1=sl[:, 4 * E:5 * E])
                    nc.vector.tensor_add(out=t64[:], in0=t64[:], in1=aslice)
                    nc.vector.tensor_scalar_mul(
                        out=osup[:, i * E:(i + 1) * E], in0=t64[:],
                        scalar1=recip[:, tt:tt + 1])
                nc.sync.dma_start(out=out_t[:, g * GT * E:(g + 1) * GT * E],
                                  in_=osup[:])
    nc.compile()
    return nc


def _make_runner(nc):
    import jax
    import concourse.mybir as mybir
    from concourse import bass2jax
    from jax.sharding import Mesh, PartitionSpec
    from jax.experimental.shard_map import shard_map

    bass2jax.install_neuronx_cc_hook()

    in_names, out_names, out_avals, zero_shapes = [], [], [], []
    partition_name = (nc.partition_id_tensor.name
                      if nc.partition_id_tensor else None)
    for alloc in nc.m.functions[0].allocations:
        if not isinstance(alloc, mybir.MemoryLocationSet):
            continue
        name = alloc.memorylocations[0].name
        if alloc.kind == "ExternalInput":
            if name != partition_name:
                in_names.append(name)
        elif alloc.kind == "ExternalOutput":
            out_names.append(name)
            shape = tuple(alloc.tensor_shape)
            dtype = mybir.dt.np(alloc.dtype)
            out_avals.append(jax.core.ShapedArray(shape, dtype))
            zero_shapes.append((shape, dtype))
    n_params = len(in_names)
    n_outs = len(out_avals)
    all_in_names = list(in_names) + list(out_names)
    if partition_name is not None:
        all_in_names.append(partition_name)
    donate = tuple(range(n_params, n_params + n_outs))

    def _body(*args):
        operands = list(args)
        if partition_name is not None:
            operands.append(bass2jax.partition_id_tensor())
        outs = bass2jax._bass_exec_p.bind(
            *operands,
            out_avals=tuple(out_avals),
            in_names=tuple(all_in_names),
            out_names=tuple(out_names),
            lowering_input_output_aliases=(),
            sim_require_finite=True,
            sim_require_nnan=True,
            nc=nc,
        )
        return tuple(outs)

    devices = jax.devices()[:NCORES]
    mesh = Mesh(np.asarray(devices), ("core",))
    specs = [PartitionSpec() if name == "table_t" else PartitionSpec("core")
             for name in in_names]
    in_specs = tuple(specs) + (PartitionSpec("core"),) * n_outs
    out_specs = (PartitionSpec("core"),) * len(out_names)
    fn = jax.jit(
        shard_map(_body, mesh=mesh, in_specs=in_specs, out_specs=out_specs,
                  check_rep=False),
        donate_argnums=donate, keep_unused=True,
    )
    return fn, mesh, in_names, out_names, zero_shapes


def _percore_sorted(keys, mask, c):
    """Per-slot valid-first key ordering for core c."""
    k = np.asarray(keys[c * BL:(c + 1) * BL]).reshape(SL, N)
    m = np.asarray(mask[c * BL:(c + 1) * BL]).reshape(SL, N) != 0
    order = np.argsort(~m, axis=1, kind="stable")
    ksort = np.take_along_axis(k, order, axis=1).astype(np.int64)
    vcnt = m.sum(axis=1)
    return ksort, vcnt, m


def needed_ocols(keys, mask):
    """Per-occurrence overflow column counts (max over cores)."""
    mx = [0] * (N - C)
    for c in range(NCORES):
        _, vcnt, _ = _percore_sorted(keys, mask, c)
        for j in range(N - C):
            cnt = int((vcnt > C + j).sum())
            mx[j] = max(mx[j], (cnt + P - 1) // P)
    while mx and mx[-1] == 0:
        mx.pop()
    return tuple(mx)


def marshal_inputs(keys, mask, ocols):
    OC = sum(ocols)
    ocw = max(OC, 1)
    skeys_g = np.empty((NCORES * P, NT * C), np.int32)
    okeys_g = np.full((NCORES * P, ocw), V, np.int32)
    osid_g = np.full((NCORES * P, ocw * 8), DUMP_SLOT, np.int16)
    mask_g = np.empty((NCORES * P, NT * N), np.int32)
    ooff = np.concatenate(([0], np.cumsum(ocols))).astype(int)
    for c in range(NCORES):
        ksort, vcnt, m = _percore_sorted(keys, mask, c)
        static = ksort[:, :C].copy()
        static[np.arange(C)[None, :] >= vcnt[:, None]] = V
        skeys_g[c * P:(c + 1) * P] = (
            static.reshape(NT, P, C).transpose(1, 0, 2)
            .reshape(P, NT * C).astype(np.int32))
        mask_g[c * P:(c + 1) * P] = (
            m.reshape(NT, P, N).transpose(1, 0, 2)
            .reshape(P, NT * N).astype(np.int32))
        for j in range(len(ocols)):
            sel = np.flatnonzero(vcnt > C + j)       # slots with occurrence j
            kj = ksort[sel, C + j]
            cj = ocols[j]
            if len(sel) > cj * P:
                raise OverflowError(f"occurrence {j}: {len(sel)} > {cj * P}")
            kflat = np.full(cj * P, V, np.int64)
            sflat = np.full(cj * P, DUMP_SLOT, np.int64)
            kflat[:len(sel)] = kj
            sflat[:len(sel)] = sel
            # source cell for stream pos i: partition i%128, col off + i//128
            okeys_g[c * P:(c + 1) * P, ooff[j]:ooff[j] + cj] = (
                kflat.reshape(cj, P).T.astype(np.int32))
            # idx wrapped layout per call: pos i -> [i%16, i//16], replicated
            w = sflat.reshape(cj * 8, 16).T.astype(np.int16)   # [16, cj*8]
            osid_g[c * P:(c + 1) * P, ooff[j] * 8:(ooff[j] + cj) * 8] = (
                np.tile(w, (8, 1)))
    return {"skeys_t": skeys_g, "okeys_t": okeys_g, "osid_t": osid_g,
            "mask_t": mask_g}


def pad_table(table):
    t = np.zeros((VPAD, E), np.float32)
    t[:V] = table
    return t


def unmarshal_output(out_g):
    out = np.empty((B, S, E), np.float32)
    for c in range(NCORES):
        oc = np.asarray(out_g[c * P:(c + 1) * P])
        out[c * BL:(c + 1) * BL] = (
            oc.reshape(P, NT, E).transpose(1, 0, 2).reshape(BL, S, E))
    return out


def _get_state(ocols):
    if _STATE.get("ocols") != ocols:
        nc = _build_nc(ocols)
        fn, mesh, in_names, out_names, zero_shapes = _make_runner(nc)
        _STATE.update(ocols=ocols, nc=nc, fn=fn, mesh=mesh,
                      in_names=in_names, out_names=out_names,
                      zero_shapes=zero_shapes, table_key=None)
    return _STATE


def kernel(keys, mask, table):
    import jax
    from jax.sharding import NamedSharding, PartitionSpec

    ocols = needed_ocols(keys, mask)
    st = _get_state(ocols)
    ins = marshal_inputs(keys, mask, ocols)

    tkey = id(table)
    if st.get("table_key") != tkey:
        st["table_dev"] = jax.device_put(
            pad_table(np.asarray(table, dtype=np.float32)),
            NamedSharding(st["mesh"], PartitionSpec()))
        st["table_key"] = tkey
    ins["table_t"] = st["table_dev"]

    args = [ins[name] for name in st["in_names"]]
    zshape, zdtype = st["zero_shapes"][0]
    zeros_out = np.zeros((NCORES * zshape[0], *zshape[1:]), zdtype)
    outs = st["fn"](*args, zeros_out)
    out_g = np.asarray(jax.block_until_ready(outs[0]))
    return unmarshal_output(out_g)
